# revision 9
# baseline (speedup 1.0000x reference)
"""Trainium2 Bass kernel for nn_CausalUpsamplingLRU.

Causal upsampling LRU: 32 autoregressive passes of a diagonal complex LRU
over a 512-long sequence, feeding each pass's output back as the next input.

Strategy:
 - Data-parallel over batch: B=8 -> one batch element per NeuronCore.
 - Feature-major layout [features, seq] per core; everything SBUF-resident.
 - Diagonal complex recurrence factored as lam = m * e^{i*theta}:
   h_t = e^{i*theta*t} * s_t with the REAL scan s_t = m*s_{t-1} + w_t,
   w_t = e^{-i*theta*t} u_t, using the HW tensor_tensor_scan (fp32 state).
 - Rotations are quadrant-packed: per (n-tile, chunk) ONE wide bf16 DVE
   multiply against a stride-0 repeated operand computes all 4 products,
   with signs folded into host-precomputed tables. Rotate-in needs one
   1024/512-wide add (GpSimd); rotate-out quadrants feed the C matmuls
   directly (adds folded into PSUM accumulation).
 - CAUSAL CHUNK PIPELINE: seq is split into chunks (CHUNK=256). Chunk c of
   pass k+1 depends only on chunk c of pass k (plus scan-state chaining
   within a pass), so there is no per-pass barrier: all engines stream.
 - Dtypes: B/D weights f32r (1 piece), x feedback f32r, elementwise chain
   (u2/t/W/S/A) bf16, C weights bf16 split hi+lo (2 pieces).  Simulated
   end-to-end rel err ~6e-3 (tolerance 2e-2).
"""
import sys
if '/opt/trn_rl_repo' not in sys.path:
    sys.path.insert(0, '/opt/trn_rl_repo')
import numpy as np
import concourse.bass as bass
import concourse.tile as tile
from concourse import bacc, mybir
from concourse.bass_types import AP
from concourse.bass_utils import run_bass_kernel_spmd

F32 = mybir.dt.float32
F32R = mybir.dt.float32r
BF16 = mybir.dt.bfloat16
OP = mybir.AluOpType
ACT_COPY = mybir.ActivationFunctionType.Copy
ACT_IDENT = mybir.ActivationFunctionType.Identity

B_SZ, SEQ, IN_CH, OUT_CH, STATE, OUT_SEQ = 8, 512, 256, 256, 384, 32
NT = STATE // 128   # 3 n-tiles
CT = IN_CH // 128   # 2 c-chunks
OT = OUT_CH // 128  # 2 o-tiles

C_PIECES = 2        # bf16 weight pieces for the C matmuls (2 = exact-ish)
CHUNK = 256         # seq chunk width for the causal pipeline
N_LOOP = 1          # hardware-loop repeats of whole program (timing only)

_BUILD_CACHE = {}


def _ap3(ap_full, off, blocks, bstride, width):
    """[128, blocks, width] view: `blocks` blocks of `width` at stride
    `bstride`, starting `off` elements into the tile."""
    return AP(ap_full.tensor, ap_full.offset + off,
              [ap_full.ap[0], [bstride, blocks], [1, width]])


def _build_nc():
    key = (C_PIECES, CHUNK, N_LOOP)
    if key in _BUILD_CACHE:
        return _BUILD_CACHE[key]
    NP = C_PIECES
    CH = CHUNK
    K = SEQ // CH
    nc = bacc.Bacc("TRN2", target_bir_lowering=False, debug=False)

    # ---- DRAM I/O ----
    xT_d = nc.dram_tensor("xT", [IN_CH, SEQ], F32, kind="ExternalInput")
    bgre_d = nc.dram_tensor("BgReT", [IN_CH, STATE], F32, kind="ExternalInput")
    bgim_d = nc.dram_tensor("BgImT", [IN_CH, STATE], F32, kind="ExternalInput")
    dt_d = nc.dram_tensor("DT", [IN_CH, OUT_CH], F32, kind="ExternalInput")
    cre_d = nc.dram_tensor("CReT", [NP, STATE, OUT_CH], mybir.dt.uint16,
                           kind="ExternalInput")
    cim_d = nc.dram_tensor("CImT", [NP, STATE, OUT_CH], mybir.dt.uint16,
                           kind="ExternalInput")
    t4i_d = nc.dram_tensor("T4IN", [STATE, 4 * SEQ], mybir.dt.uint16,
                           kind="ExternalInput")
    t4o_d = nc.dram_tensor("T4OUT", [STATE, 4 * SEQ], mybir.dt.uint16,
                           kind="ExternalInput")
    mcol_d = nc.dram_tensor("MCOL", [STATE, 1], F32, kind="ExternalInput")
    c512_d = nc.dram_tensor("C512", [STATE, 1], F32, kind="ExternalInput")
    s512_d = nc.dram_tensor("S512", [STATE, 1], F32, kind="ExternalInput")
    s512n_d = nc.dram_tensor("S512N", [STATE, 1], F32, kind="ExternalInput")
    out_d = nc.dram_tensor("OUT", [OUT_CH, OUT_SEQ], F32, kind="ExternalOutput")

    with tile.TileContext(nc) as tc:
        with tc.tile_pool(name="const", bufs=1) as cp, \
             tc.tile_pool(name="xp", bufs=6) as xp, \
             tc.tile_pool(name="up", bufs=4, space="PSUM") as up, \
             tc.tile_pool(name="yp", bufs=2, space="PSUM") as yp, \
             tc.tile_pool(name="u2p", bufs=6) as u2p, \
             tc.tile_pool(name="tp", bufs=4) as tp, \
             tc.tile_pool(name="wp", bufs=4) as wp, \
             tc.tile_pool(name="sp", bufs=6) as sp, \
             tc.tile_pool(name="apool", bufs=4) as apool, \
             tc.tile_pool(name="cyp", bufs=24) as cyp:

            # ---- persistent constants ----
            bgre = [cp.tile([128, STATE], F32R, tag=f"bgre{j}", name=f"bgre{j}")
                    for j in range(CT)]
            bgim = [cp.tile([128, STATE], F32R, tag=f"bgim{j}", name=f"bgim{j}")
                    for j in range(CT)]
            dtw = [cp.tile([128, OUT_CH], F32R, tag=f"dtw{j}", name=f"dtw{j}")
                   for j in range(CT)]
            cre = [[cp.tile([128, OUT_CH], BF16, tag=f"cre{p}{j}", name=f"cre{p}{j}")
                    for j in range(NT)] for p in range(NP)]
            cim = [[cp.tile([128, OUT_CH], BF16, tag=f"cim{p}{j}", name=f"cim{p}{j}")
                    for j in range(NT)] for p in range(NP)]
            t4i = [cp.tile([128, 4 * SEQ], BF16, tag=f"t4i{j}", name=f"t4i{j}")
                   for j in range(NT)]
            t4o = [cp.tile([128, 4 * SEQ], BF16, tag=f"t4o{j}", name=f"t4o{j}")
                   for j in range(NT)]
            mcol = [cp.tile([128, 1], F32, tag=f"mcol{j}", name=f"mcol{j}") for j in range(NT)]
            c512 = [cp.tile([128, 1], F32, tag=f"c512{j}", name=f"c512{j}") for j in range(NT)]
            s512 = [cp.tile([128, 1], F32, tag=f"s512{j}", name=f"s512{j}") for j in range(NT)]
            s512n = [cp.tile([128, 1], F32, tag=f"s512n{j}", name=f"s512n{j}") for j in range(NT)]
            outb = [cp.tile([128, OUT_SEQ], F32, tag=f"outb{j}", name=f"outb{j}") for j in range(OT)]

            for j in range(CT):
                nc.sync.dma_start(out=bgre[j][:], in_=bgre_d[j*128:(j+1)*128, :].bitcast(F32R))
                nc.sync.dma_start(out=bgim[j][:], in_=bgim_d[j*128:(j+1)*128, :].bitcast(F32R))
                nc.sync.dma_start(out=dtw[j][:], in_=dt_d[j*128:(j+1)*128, :].bitcast(F32R))
            for p in range(NP):
                for j in range(NT):
                    nc.sync.dma_start(out=cre[p][j][:], in_=cre_d[p, j*128:(j+1)*128, :].bitcast(BF16))
                    nc.sync.dma_start(out=cim[p][j][:], in_=cim_d[p, j*128:(j+1)*128, :].bitcast(BF16))
            for j in range(NT):
                nc.sync.dma_start(out=t4i[j][:], in_=t4i_d[j*128:(j+1)*128, :].bitcast(BF16))
                nc.sync.dma_start(out=t4o[j][:], in_=t4o_d[j*128:(j+1)*128, :].bitcast(BF16))
                nc.sync.dma_start(out=mcol[j][:], in_=mcol_d[j*128:(j+1)*128, :])
                nc.sync.dma_start(out=c512[j][:], in_=c512_d[j*128:(j+1)*128, :])
                nc.sync.dma_start(out=s512[j][:], in_=s512_d[j*128:(j+1)*128, :])
                nc.sync.dma_start(out=s512n[j][:], in_=s512n_d[j*128:(j+1)*128, :])

            def emit_body():
                # x chunks: xa[h][jc] = [128, CH] f32r
                xa = [[xp.tile([128, CH], F32R, tag=f"x{h}{jc}", name=f"x{h}{jc}")
                       for jc in range(CT)] for h in range(K)]
                for h in range(K):
                    for jc in range(CT):
                        nc.sync.dma_start(
                            out=xa[h][jc][:],
                            in_=xT_d[jc*128:(jc+1)*128, h*CH:(h+1)*CH].bitcast(F32R))

                carry_re = [None] * NT
                carry_im = [None] * NT
                s_prev = [None] * NT     # previous chunk's S tile (scan chain)
                stages = [(it, h) for it in range(OUT_SEQ) for h in range(K)]

                def emit_U(stage):
                    """U matmuls + PSUM->SBUF bf16 eviction for one stage."""
                    it, h = stage
                    u2s = []
                    for jn in range(NT):
                        u = up.tile([128, 2 * CH], F32, tag="u", name="u")
                        for jc in range(CT):
                            nc.tensor.matmul(u[:, 0:CH],
                                             bgre[jc][:, jn*128:(jn+1)*128],
                                             xa[h][jc][:], start=(jc == 0),
                                             stop=(jc == CT-1))
                        for jc in range(CT):
                            nc.tensor.matmul(u[:, CH:2*CH],
                                             bgim[jc][:, jn*128:(jn+1)*128],
                                             xa[h][jc][:], start=(jc == 0),
                                             stop=(jc == CT-1))
                        u2 = u2p.tile([128, 2 * CH], BF16, tag="u2", name="u2")
                        nc.scalar.activation(u2[:], u[:], ACT_COPY)
                        u2s.append(u2)
                    return u2s

                def emit_V(stage, u2s):
                    """rotate-in, W, scans, rotate-out, carries for one stage."""
                    nonlocal s_prev, carry_re, carry_im
                    it, h = stage
                    co = h * CH
                    # rotate-in quadrant multiplies (V): T4IN quadrants
                    # [cos | sin | -sin | cos] -> [c*ur | s*ui | -s*ur | c*ui]
                    talls = []
                    for jn in range(NT):
                        tall = tp.tile([128, 4 * CH], BF16, tag="t", name="t")
                        tbl = _ap3(t4i[jn][:], co, 4, SEQ, CH)
                        rep = _ap3(u2s[jn][:], 0, 2, 0, 2 * CH)
                        nc.vector.tensor_tensor(tall[:], tbl, rep, OP.mult)
                        # W = [q0+q1 | q3+q2] = [w_re | w_im] on GpSimd
                        w = wp.tile([128, 2 * CH], BF16, tag="w", name="w")
                        nc.gpsimd.tensor_tensor(
                            w[:],
                            _ap3(tall[:], 0, 2, 3 * CH, CH),
                            _ap3(tall[:], CH, 2, CH, CH),
                            OP.add)
                        talls.append(w)
                    # scans + rotate-out, V never waits on GpSimd
                    aqs = []
                    s_cur = [None] * NT
                    for jn in range(NT):
                        w = talls[jn]
                        s = sp.tile([128, 2 * CH], BF16, tag="s", name="s")
                        if h == 0:
                            init_r = 0.0 if it == 0 else carry_re[jn][:]
                            init_i = 0.0 if it == 0 else carry_im[jn][:]
                        else:
                            init_r = s_prev[jn][:, CH-1:CH]
                            init_i = s_prev[jn][:, 2*CH-1:2*CH]
                        d0 = mcol[jn][:].broadcast_to((128, CH))
                        nc.vector.tensor_tensor_scan(s[:, 0:CH], d0, w[:, 0:CH],
                                                     init_r, OP.mult, OP.add)
                        nc.vector.tensor_tensor_scan(s[:, CH:2*CH], d0, w[:, CH:2*CH],
                                                     init_i, OP.mult, OP.add)
                        s_cur[jn] = s
                        a = apool.tile([128, 4 * CH], BF16, tag="a", name="a")
                        tbl = _ap3(t4o[jn][:], co, 4, SEQ, CH)
                        rep = _ap3(s[:], 0, 2, 0, 2 * CH)
                        nc.vector.tensor_tensor(a[:], tbl, rep, OP.mult)
                        aqs.append(a)
                        # pass-boundary carry (Scalar engine; TensorScalarPtr
                        # is not implemented on Pool): carry = e^{i*512th}*s_end
                        if h == K - 1 and it < OUT_SEQ - 1:
                            q = cyp.tile([128, 1], F32, tag="cy", name="cy")
                            crn = cyp.tile([128, 1], F32, tag="cy", name="cy")
                            nc.scalar.activation(q[:], s[:, 2*CH-1:2*CH],
                                                 ACT_COPY, scale=s512n[jn][:])
                            nc.scalar.activation(crn[:], s[:, CH-1:CH],
                                                 ACT_IDENT, scale=c512[jn][:],
                                                 bias=q[:])
                            r2 = cyp.tile([128, 1], F32, tag="cy", name="cy")
                            cin = cyp.tile([128, 1], F32, tag="cy", name="cy")
                            nc.scalar.activation(r2[:], s[:, CH-1:CH],
                                                 ACT_COPY, scale=s512[jn][:])
                            nc.scalar.activation(cin[:], s[:, 2*CH-1:2*CH],
                                                 ACT_IDENT, scale=c512[jn][:],
                                                 bias=r2[:])
                            carry_re[jn] = crn
                            carry_im[jn] = cin
                    s_prev = s_cur
                    return aqs

                def emit_C(stage, aqs):
                    """D + C matmuls, x feedback copies, output column."""
                    it, h = stage
                    xa_next = [None] * CT
                    for jo in range(OT):
                        o0 = jo * 128
                        y = yp.tile([128, CH], F32, tag=f"y{jo}", name=f"y{jo}")
                        for jc in range(CT):
                            nc.tensor.matmul(y[:], dtw[jc][:, o0:o0+128],
                                             xa[h][jc][:], start=(jc == 0),
                                             stop=False)
                        for jn in range(NT):
                            a = aqs[jn]
                            for p in range(NP):
                                last = (jn == NT-1 and p == NP-1)
                                nc.tensor.matmul(y[:], cre[p][jn][:, o0:o0+128],
                                                 a[:, 0:CH], start=False, stop=False)
                                nc.tensor.matmul(y[:], cre[p][jn][:, o0:o0+128],
                                                 a[:, CH:2*CH], start=False, stop=False)
                                nc.tensor.matmul(y[:], cim[p][jn][:, o0:o0+128],
                                                 a[:, 2*CH:3*CH], start=False, stop=False)
                                nc.tensor.matmul(y[:], cim[p][jn][:, o0:o0+128],
                                                 a[:, 3*CH:4*CH], start=False, stop=last)
                        if it < OUT_SEQ - 1:
                            xn = xp.tile([128, CH], F32R, tag=f"x{h}{jo}",
                                         name=f"xn{h}{jo}")
                            nc.scalar.activation(xn[:], y[:], ACT_COPY)
                            xa_next[jo] = xn
                        if h == K - 1:
                            nc.scalar.activation(outb[jo][:, it:it+1],
                                                 y[:, CH-1:CH], ACT_COPY)
                    if it < OUT_SEQ - 1:
                        xa[h] = xa_next

                # software-pipelined emission: U/u2 of stage s+1 land in the
                # PE/Scalar queues BEFORE the C-block of stage s, so PE feeds
                # the V ladder of s+1 while it chews on C MMs of stage s.
                # (Requires K >= 2: U(s+1) reads x written at stage s-1.)
                assert K >= 2, "pipelined emission needs CHUNK <= 256"
                pend = emit_U(stages[0])
                for idx, stage in enumerate(stages):
                    nxt = emit_U(stages[idx + 1]) if idx + 1 < len(stages) else None
                    art = emit_V(stage, pend)
                    emit_C(stage, art)
                    pend = nxt
            if N_LOOP > 1:
                with tc.For_i(0, N_LOOP, 1) as _i:
                    emit_body()
            else:
                emit_body()

            for jo in range(OT):
                nc.sync.dma_start(out=out_d[jo*128:(jo+1)*128, :], in_=outb[jo][:])
    nc.compile()
    _BUILD_CACHE[key] = nc
    return nc


def _round_f32r(a):
    u = np.ascontiguousarray(a.astype(np.float32)).view(np.uint32)
    r = ((u.astype(np.uint64) + (1 << 11)) >> 12 << 12).astype(np.uint32)
    return r.view(np.float32)


def _to_bf16_bits(a):
    u = np.ascontiguousarray(a.astype(np.float32)).view(np.uint32)
    r = ((u.astype(np.uint64) + 0x7fff + ((u >> 16) & 1)) >> 16).astype(np.uint32)
    return r.astype(np.uint16)


def _bf16_val(bits):
    return (bits.astype(np.uint32) << 16).view(np.float32)


def _host_precompute(x, nu_log, theta_log, gamma_log, B_re, B_im, C_re, C_im, D):
    f8 = np.float64
    nu_log = np.asarray(nu_log, f8); theta_log = np.asarray(theta_log, f8)
    gamma_log = np.asarray(gamma_log, f8)
    B_re = np.asarray(B_re, f8); B_im = np.asarray(B_im, f8)
    C_re = np.asarray(C_re, f8); C_im = np.asarray(C_im, f8)
    D = np.asarray(D, f8)
    m = np.exp(-np.exp(nu_log)); theta = np.exp(theta_log)
    gamma = np.exp(gamma_log)
    t = np.arange(1, SEQ + 1, dtype=f8)[None, :]
    ang = theta[:, None] * t
    f4 = np.float32
    cos = np.cos(ang).astype(f4); sin = np.sin(ang).astype(f4)

    T4IN = np.concatenate([cos, sin, -sin, cos], axis=1)       # [STATE, 2048]
    T4OUT = np.concatenate([cos, -sin, -sin, -cos], axis=1)

    def pieces_bf16(w):
        wT = np.ascontiguousarray(w.T.astype(f4))
        hi_bits = _to_bf16_bits(wT)
        if C_PIECES == 1:
            return hi_bits[None]
        lo_bits = _to_bf16_bits(wT - _bf16_val(hi_bits))
        return np.stack([hi_bits, lo_bits], axis=0)

    common = dict(
        BgReT=_round_f32r(np.ascontiguousarray((gamma[:, None] * B_re).T)),
        BgImT=_round_f32r(np.ascontiguousarray((gamma[:, None] * B_im).T)),
        DT=_round_f32r(np.ascontiguousarray(D.T)),
        CReT=pieces_bf16(C_re),
        CImT=pieces_bf16(C_im),
        T4IN=_to_bf16_bits(T4IN),
        T4OUT=_to_bf16_bits(T4OUT),
        MCOL=m.astype(f4)[:, None],
        C512=np.cos(theta * SEQ).astype(f4)[:, None],
        S512=np.sin(theta * SEQ).astype(f4)[:, None],
        S512N=(-np.sin(theta * SEQ)).astype(f4)[:, None],
    )
    x = np.asarray(x, np.float32)
    in_maps = []
    for b in range(B_SZ):
        im = dict(common)
        im['xT'] = np.ascontiguousarray(x[b].T)
        in_maps.append(im)
    return in_maps


def kernel(x, nu_log, theta_log, gamma_log, B_re, B_im, C_re, C_im, D):
    nc = _build_nc()
    in_maps = _host_precompute(x, nu_log, theta_log, gamma_log,
                               B_re, B_im, C_re, C_im, D)
    res = run_bass_kernel_spmd(nc, in_maps, list(range(B_SZ)))
    out = np.stack([res.results[b]['OUT'].T for b in range(B_SZ)], axis=0)
    return np.ascontiguousarray(out.astype(np.float32))


# revision 11
# speedup vs baseline: 37.0756x; 37.0756x over previous
"""Trainium2 Bass kernel for nn_CausalUpsamplingLRU.

Causal upsampling LRU: 32 autoregressive passes of a diagonal complex LRU
over a 512-long sequence, feeding each pass's output back as the next input.

Strategy:
 - Data-parallel over batch: B=8 -> one batch element per NeuronCore.
 - Feature-major layout [features, seq] per core; everything SBUF-resident.
 - Diagonal complex recurrence factored as lam = m * e^{i*theta}:
   h_t = e^{i*theta*t} * s_t with the REAL scan s_t = m*s_{t-1} + w_t,
   w_t = e^{-i*theta*t} u_t, using the HW tensor_tensor_scan (fp32 state).
 - Rotations are quadrant-packed: per (n-tile, chunk) ONE wide bf16 DVE
   multiply against a stride-0 repeated operand computes all 4 products,
   with signs folded into host-precomputed tables. Rotate-in needs one
   1024/512-wide add (GpSimd); rotate-out quadrants feed the C matmuls
   directly (adds folded into PSUM accumulation).
 - CAUSAL CHUNK PIPELINE: seq is split into chunks (CHUNK=256). Chunk c of
   pass k+1 depends only on chunk c of pass k (plus scan-state chaining
   within a pass), so there is no per-pass barrier: all engines stream.
 - Dtypes: B/D weights f32r (1 piece), x feedback f32r, elementwise chain
   (u2/t/W/S/A) bf16, C weights bf16 split hi+lo (2 pieces).  Simulated
   end-to-end rel err ~6e-3 (tolerance 2e-2).
"""
import sys
if '/opt/trn_rl_repo' not in sys.path:
    sys.path.insert(0, '/opt/trn_rl_repo')
import numpy as np
import concourse.bass as bass
import concourse.tile as tile
from concourse import bacc, mybir
from concourse.bass_types import AP
from concourse.bass_utils import run_bass_kernel_spmd

F32 = mybir.dt.float32
F32R = mybir.dt.float32r
BF16 = mybir.dt.bfloat16
OP = mybir.AluOpType
ACT_COPY = mybir.ActivationFunctionType.Copy
ACT_IDENT = mybir.ActivationFunctionType.Identity

B_SZ, SEQ, IN_CH, OUT_CH, STATE, OUT_SEQ = 8, 512, 256, 256, 384, 32
NT = STATE // 128   # 3 n-tiles
CT = IN_CH // 128   # 2 c-chunks
OT = OUT_CH // 128  # 2 o-tiles

C_PIECES = 2        # bf16 weight pieces for the C matmuls (2 = exact-ish)
CHUNK = 256         # seq chunk width for the causal pipeline
N_LOOP = 1          # hardware-loop repeats of whole program (timing only)

_BUILD_CACHE = {}


def _ap3(ap_full, off, blocks, bstride, width):
    """[128, blocks, width] view: `blocks` blocks of `width` at stride
    `bstride`, starting `off` elements into the tile."""
    return AP(ap_full.tensor, ap_full.offset + off,
              [ap_full.ap[0], [bstride, blocks], [1, width]])


def _build_nc():
    key = (C_PIECES, CHUNK, N_LOOP)
    if key in _BUILD_CACHE:
        return _BUILD_CACHE[key]
    NP = C_PIECES
    CH = CHUNK
    K = SEQ // CH
    nc = bacc.Bacc("TRN2", target_bir_lowering=False, debug=False)

    # ---- DRAM I/O ----
    xT_d = nc.dram_tensor("xT", [IN_CH, SEQ], F32, kind="ExternalInput")
    bgre_d = nc.dram_tensor("BgReT", [IN_CH, STATE], F32, kind="ExternalInput")
    bgim_d = nc.dram_tensor("BgImT", [IN_CH, STATE], F32, kind="ExternalInput")
    dt_d = nc.dram_tensor("DT", [IN_CH, OUT_CH], F32, kind="ExternalInput")
    cre_d = nc.dram_tensor("CReT", [NP, STATE, OUT_CH], mybir.dt.uint16,
                           kind="ExternalInput")
    cim_d = nc.dram_tensor("CImT", [NP, STATE, OUT_CH], mybir.dt.uint16,
                           kind="ExternalInput")
    t4i_d = nc.dram_tensor("T4IN", [STATE, 4 * SEQ], mybir.dt.uint16,
                           kind="ExternalInput")
    t4o_d = nc.dram_tensor("T4OUT", [STATE, 4 * SEQ], mybir.dt.uint16,
                           kind="ExternalInput")
    mcol_d = nc.dram_tensor("MCOL", [STATE, 1], F32, kind="ExternalInput")
    c512_d = nc.dram_tensor("C512", [STATE, 1], F32, kind="ExternalInput")
    s512_d = nc.dram_tensor("S512", [STATE, 1], F32, kind="ExternalInput")
    s512n_d = nc.dram_tensor("S512N", [STATE, 1], F32, kind="ExternalInput")
    out_d = nc.dram_tensor("OUT", [OUT_CH, OUT_SEQ], F32, kind="ExternalOutput")

    with tile.TileContext(nc) as tc:
        with tc.tile_pool(name="const", bufs=1) as cp, \
             tc.tile_pool(name="xp", bufs=8) as xp, \
             tc.tile_pool(name="up", bufs=4, space="PSUM") as up, \
             tc.tile_pool(name="yp", bufs=2, space="PSUM") as yp, \
             tc.tile_pool(name="u2p", bufs=8) as u2p, \
             tc.tile_pool(name="tp", bufs=6) as tp, \
             tc.tile_pool(name="wp", bufs=6) as wp, \
             tc.tile_pool(name="sp", bufs=8) as sp, \
             tc.tile_pool(name="apool", bufs=6) as apool, \
             tc.tile_pool(name="cyp", bufs=24) as cyp:

            # ---- persistent constants ----
            bgre = [cp.tile([128, STATE], F32R, tag=f"bgre{j}", name=f"bgre{j}")
                    for j in range(CT)]
            bgim = [cp.tile([128, STATE], F32R, tag=f"bgim{j}", name=f"bgim{j}")
                    for j in range(CT)]
            dtw = [cp.tile([128, OUT_CH], F32R, tag=f"dtw{j}", name=f"dtw{j}")
                   for j in range(CT)]
            cre = [[cp.tile([128, OUT_CH], BF16, tag=f"cre{p}{j}", name=f"cre{p}{j}")
                    for j in range(NT)] for p in range(NP)]
            cim = [[cp.tile([128, OUT_CH], BF16, tag=f"cim{p}{j}", name=f"cim{p}{j}")
                    for j in range(NT)] for p in range(NP)]
            t4i = [cp.tile([128, 4 * SEQ], BF16, tag=f"t4i{j}", name=f"t4i{j}")
                   for j in range(NT)]
            t4o = [cp.tile([128, 4 * SEQ], BF16, tag=f"t4o{j}", name=f"t4o{j}")
                   for j in range(NT)]
            mcol = [cp.tile([128, 1], F32, tag=f"mcol{j}", name=f"mcol{j}") for j in range(NT)]
            c512 = [cp.tile([128, 1], F32, tag=f"c512{j}", name=f"c512{j}") for j in range(NT)]
            s512 = [cp.tile([128, 1], F32, tag=f"s512{j}", name=f"s512{j}") for j in range(NT)]
            s512n = [cp.tile([128, 1], F32, tag=f"s512n{j}", name=f"s512n{j}") for j in range(NT)]
            outb = [cp.tile([128, OUT_SEQ], F32, tag=f"outb{j}", name=f"outb{j}") for j in range(OT)]

            for j in range(CT):
                nc.sync.dma_start(out=bgre[j][:], in_=bgre_d[j*128:(j+1)*128, :].bitcast(F32R))
                nc.sync.dma_start(out=bgim[j][:], in_=bgim_d[j*128:(j+1)*128, :].bitcast(F32R))
                nc.sync.dma_start(out=dtw[j][:], in_=dt_d[j*128:(j+1)*128, :].bitcast(F32R))
            for p in range(NP):
                for j in range(NT):
                    nc.sync.dma_start(out=cre[p][j][:], in_=cre_d[p, j*128:(j+1)*128, :].bitcast(BF16))
                    nc.sync.dma_start(out=cim[p][j][:], in_=cim_d[p, j*128:(j+1)*128, :].bitcast(BF16))
            for j in range(NT):
                nc.sync.dma_start(out=t4i[j][:], in_=t4i_d[j*128:(j+1)*128, :].bitcast(BF16))
                nc.sync.dma_start(out=t4o[j][:], in_=t4o_d[j*128:(j+1)*128, :].bitcast(BF16))
                nc.sync.dma_start(out=mcol[j][:], in_=mcol_d[j*128:(j+1)*128, :])
                nc.sync.dma_start(out=c512[j][:], in_=c512_d[j*128:(j+1)*128, :])
                nc.sync.dma_start(out=s512[j][:], in_=s512_d[j*128:(j+1)*128, :])
                nc.sync.dma_start(out=s512n[j][:], in_=s512n_d[j*128:(j+1)*128, :])

            def emit_body():
                # x chunks: xa[h][jc] = [128, CH] f32r
                xa = [[xp.tile([128, CH], F32R, tag=f"x{h}{jc}", name=f"x{h}{jc}")
                       for jc in range(CT)] for h in range(K)]
                for h in range(K):
                    for jc in range(CT):
                        nc.sync.dma_start(
                            out=xa[h][jc][:],
                            in_=xT_d[jc*128:(jc+1)*128, h*CH:(h+1)*CH].bitcast(F32R))

                carry_re = [None] * NT
                carry_im = [None] * NT
                s_prev = [None] * NT     # previous chunk's S tile (scan chain)
                stages = [(it, h) for it in range(OUT_SEQ) for h in range(K)]

                def emit_U(stage):
                    """U matmuls + PSUM->SBUF bf16 eviction for one stage."""
                    it, h = stage
                    u2s = []
                    for jn in range(NT):
                        u = up.tile([128, 2 * CH], F32, tag="u", name="u")
                        for jc in range(CT):
                            nc.tensor.matmul(u[:, 0:CH],
                                             bgre[jc][:, jn*128:(jn+1)*128],
                                             xa[h][jc][:], start=(jc == 0),
                                             stop=(jc == CT-1))
                        for jc in range(CT):
                            nc.tensor.matmul(u[:, CH:2*CH],
                                             bgim[jc][:, jn*128:(jn+1)*128],
                                             xa[h][jc][:], start=(jc == 0),
                                             stop=(jc == CT-1))
                        u2 = u2p.tile([128, 2 * CH], BF16, tag="u2", name="u2")
                        nc.scalar.activation(u2[:], u[:], ACT_COPY)
                        u2s.append(u2)
                    return u2s

                def emit_V(stage, u2s):
                    """rotate-in, W, scans, rotate-out, carries for one stage."""
                    nonlocal s_prev, carry_re, carry_im
                    it, h = stage
                    co = h * CH
                    # rotate-in quadrant multiplies (V): T4IN quadrants
                    # [cos | sin | -sin | cos] -> [c*ur | s*ui | -s*ur | c*ui]
                    talls = []
                    for jn in range(NT):
                        tall = tp.tile([128, 4 * CH], BF16, tag="t", name="t")
                        tbl = _ap3(t4i[jn][:], co, 4, SEQ, CH)
                        rep = _ap3(u2s[jn][:], 0, 2, 0, 2 * CH)
                        nc.vector.tensor_tensor(tall[:], tbl, rep, OP.mult)
                        # W = [q0+q1 | q3+q2] = [w_re | w_im] on GpSimd
                        w = wp.tile([128, 2 * CH], BF16, tag="w", name="w")
                        nc.gpsimd.tensor_tensor(
                            w[:],
                            _ap3(tall[:], 0, 2, 3 * CH, CH),
                            _ap3(tall[:], CH, 2, CH, CH),
                            OP.add)
                        talls.append(w)
                    # scans + rotate-out, V never waits on GpSimd
                    aqs = []
                    s_cur = [None] * NT
                    for jn in range(NT):
                        w = talls[jn]
                        s = sp.tile([128, 2 * CH], BF16, tag="s", name="s")
                        if h == 0:
                            init_r = 0.0 if it == 0 else carry_re[jn][:]
                            init_i = 0.0 if it == 0 else carry_im[jn][:]
                        else:
                            init_r = s_prev[jn][:, CH-1:CH]
                            init_i = s_prev[jn][:, 2*CH-1:2*CH]
                        d0 = mcol[jn][:].broadcast_to((128, CH))
                        nc.vector.tensor_tensor_scan(s[:, 0:CH], d0, w[:, 0:CH],
                                                     init_r, OP.mult, OP.add)
                        nc.vector.tensor_tensor_scan(s[:, CH:2*CH], d0, w[:, CH:2*CH],
                                                     init_i, OP.mult, OP.add)
                        s_cur[jn] = s
                        a = apool.tile([128, 4 * CH], BF16, tag="a", name="a")
                        tbl = _ap3(t4o[jn][:], co, 4, SEQ, CH)
                        rep = _ap3(s[:], 0, 2, 0, 2 * CH)
                        # A0 runs on GpSimd (queued behind the W's) so V's
                        # scan+rotate-out ladder and GpSimd finish together
                        eng = nc.gpsimd if jn == 0 else nc.vector
                        eng.tensor_tensor(a[:], tbl, rep, OP.mult)
                        aqs.append(a)
                        # pass-boundary carry (Scalar engine; TensorScalarPtr
                        # is not implemented on Pool): carry = e^{i*512th}*s_end
                        if h == K - 1 and it < OUT_SEQ - 1:
                            q = cyp.tile([128, 1], F32, tag="cy", name="cy")
                            crn = cyp.tile([128, 1], F32, tag="cy", name="cy")
                            nc.scalar.activation(q[:], s[:, 2*CH-1:2*CH],
                                                 ACT_COPY, scale=s512n[jn][:])
                            nc.scalar.activation(crn[:], s[:, CH-1:CH],
                                                 ACT_IDENT, scale=c512[jn][:],
                                                 bias=q[:])
                            r2 = cyp.tile([128, 1], F32, tag="cy", name="cy")
                            cin = cyp.tile([128, 1], F32, tag="cy", name="cy")
                            nc.scalar.activation(r2[:], s[:, CH-1:CH],
                                                 ACT_COPY, scale=s512[jn][:])
                            nc.scalar.activation(cin[:], s[:, 2*CH-1:2*CH],
                                                 ACT_IDENT, scale=c512[jn][:],
                                                 bias=r2[:])
                            carry_re[jn] = crn
                            carry_im[jn] = cin
                    s_prev = s_cur
                    return aqs

                def emit_C(stage, aqs):
                    """D + C matmuls, x feedback copies, output column."""
                    it, h = stage
                    xa_next = [None] * CT
                    for jo in range(OT):
                        o0 = jo * 128
                        y = yp.tile([128, CH], F32, tag=f"y{jo}", name=f"y{jo}")
                        for jc in range(CT):
                            nc.tensor.matmul(y[:], dtw[jc][:, o0:o0+128],
                                             xa[h][jc][:], start=(jc == 0),
                                             stop=False)
                        for jn in range(NT):
                            a = aqs[jn]
                            for p in range(NP):
                                last = (jn == NT-1 and p == NP-1)
                                nc.tensor.matmul(y[:], cre[p][jn][:, o0:o0+128],
                                                 a[:, 0:CH], start=False, stop=False)
                                nc.tensor.matmul(y[:], cre[p][jn][:, o0:o0+128],
                                                 a[:, CH:2*CH], start=False, stop=False)
                                nc.tensor.matmul(y[:], cim[p][jn][:, o0:o0+128],
                                                 a[:, 2*CH:3*CH], start=False, stop=False)
                                nc.tensor.matmul(y[:], cim[p][jn][:, o0:o0+128],
                                                 a[:, 3*CH:4*CH], start=False, stop=last)
                        if it < OUT_SEQ - 1:
                            xn = xp.tile([128, CH], F32R, tag=f"x{h}{jo}",
                                         name=f"xn{h}{jo}")
                            nc.scalar.activation(xn[:], y[:], ACT_COPY)
                            xa_next[jo] = xn
                        if h == K - 1:
                            nc.scalar.activation(outb[jo][:, it:it+1],
                                                 y[:, CH-1:CH], ACT_COPY)
                    if it < OUT_SEQ - 1:
                        xa[h] = xa_next

                # software-pipelined emission: U/u2 of stage s+1 land in the
                # PE/Scalar queues BEFORE the C-block of stage s, so PE feeds
                # the V ladder of s+1 while it chews on C MMs of stage s.
                # (Requires K >= 2: U(s+1) reads x written at stage s-1.)
                assert K >= 2, "pipelined emission needs CHUNK <= 256"
                pend = emit_U(stages[0])
                for idx, stage in enumerate(stages):
                    nxt = emit_U(stages[idx + 1]) if idx + 1 < len(stages) else None
                    art = emit_V(stage, pend)
                    emit_C(stage, art)
                    pend = nxt
            if N_LOOP > 1:
                with tc.For_i(0, N_LOOP, 1) as _i:
                    emit_body()
            else:
                emit_body()

            for jo in range(OT):
                nc.sync.dma_start(out=out_d[jo*128:(jo+1)*128, :], in_=outb[jo][:])
    nc.compile()
    _BUILD_CACHE[key] = nc
    return nc


def _round_f32r(a):
    u = np.ascontiguousarray(a.astype(np.float32)).view(np.uint32)
    r = ((u.astype(np.uint64) + (1 << 11)) >> 12 << 12).astype(np.uint32)
    return r.view(np.float32)


def _to_bf16_bits(a):
    u = np.ascontiguousarray(a.astype(np.float32)).view(np.uint32)
    r = ((u.astype(np.uint64) + 0x7fff + ((u >> 16) & 1)) >> 16).astype(np.uint32)
    return r.astype(np.uint16)


def _bf16_val(bits):
    return (bits.astype(np.uint32) << 16).view(np.float32)


def _host_precompute(x, nu_log, theta_log, gamma_log, B_re, B_im, C_re, C_im, D):
    f8 = np.float64
    nu_log = np.asarray(nu_log, f8); theta_log = np.asarray(theta_log, f8)
    gamma_log = np.asarray(gamma_log, f8)
    B_re = np.asarray(B_re, f8); B_im = np.asarray(B_im, f8)
    C_re = np.asarray(C_re, f8); C_im = np.asarray(C_im, f8)
    D = np.asarray(D, f8)
    m = np.exp(-np.exp(nu_log)); theta = np.exp(theta_log)
    gamma = np.exp(gamma_log)
    t = np.arange(1, SEQ + 1, dtype=f8)[None, :]
    ang = theta[:, None] * t
    f4 = np.float32
    cos = np.cos(ang).astype(f4); sin = np.sin(ang).astype(f4)

    T4IN = np.concatenate([cos, sin, -sin, cos], axis=1)       # [STATE, 2048]
    T4OUT = np.concatenate([cos, -sin, -sin, -cos], axis=1)

    def pieces_bf16(w):
        wT = np.ascontiguousarray(w.T.astype(f4))
        hi_bits = _to_bf16_bits(wT)
        if C_PIECES == 1:
            return hi_bits[None]
        lo_bits = _to_bf16_bits(wT - _bf16_val(hi_bits))
        return np.stack([hi_bits, lo_bits], axis=0)

    common = dict(
        BgReT=_round_f32r(np.ascontiguousarray((gamma[:, None] * B_re).T)),
        BgImT=_round_f32r(np.ascontiguousarray((gamma[:, None] * B_im).T)),
        DT=_round_f32r(np.ascontiguousarray(D.T)),
        CReT=pieces_bf16(C_re),
        CImT=pieces_bf16(C_im),
        T4IN=_to_bf16_bits(T4IN),
        T4OUT=_to_bf16_bits(T4OUT),
        MCOL=m.astype(f4)[:, None],
        C512=np.cos(theta * SEQ).astype(f4)[:, None],
        S512=np.sin(theta * SEQ).astype(f4)[:, None],
        S512N=(-np.sin(theta * SEQ)).astype(f4)[:, None],
    )
    x = np.asarray(x, np.float32)
    in_maps = []
    for b in range(B_SZ):
        im = dict(common)
        im['xT'] = np.ascontiguousarray(x[b].T)
        in_maps.append(im)
    return in_maps


def kernel(x, nu_log, theta_log, gamma_log, B_re, B_im, C_re, C_im, D):
    nc = _build_nc()
    in_maps = _host_precompute(x, nu_log, theta_log, gamma_log,
                               B_re, B_im, C_re, C_im, D)
    res = run_bass_kernel_spmd(nc, in_maps, list(range(B_SZ)))
    out = np.stack([res.results[b]['OUT'].T for b in range(B_SZ)], axis=0)
    return np.ascontiguousarray(out.astype(np.float32))
